# revision 46
# baseline (speedup 1.0000x reference)
"""Fused DoubleXLSTMDown kernel for 8 Trainium2 NeuronCores.

Sharding: data-parallel over batch (B=8 -> 1 batch item per core). Weights are
broadcast; the host pre-casts the big projection weights to fp8-e4m3 (with
per-tensor power-2 scales) and pre-rearranges into DMA-friendly, matmul-ready
layouts. No collectives.

On-chip layout: activations are kept feature-major [feat, seq] so every
projection is `out = W.T @ act` with W in its natural layout as lhsT; the
attention value/output path runs seq-major (group-norm needs free-dim stats).
All large projections and the attention score/SV matmuls run as fp8 DoubleRow
matmuls (2 contraction chunks of 128 per instruction, ~2x TensorE throughput);
activation operands are stored e4m3 with power-2 scales that are exactly
compensated through drain scales / decay-bias constants.

The mLSTM decay matrix uses the exact decomposition
    D[t,s] = exp(a[s] - M[t]) = exp(a[s]-G) * exp(G-M[t]),
    a = ipre + spc,  spc = cumsum(softplus(-fpre)),  M = runmax(a), G = max(a)
so no [S,S] row-max/exp is needed: u=exp(a-G-du) folds into the score-tile
copy (du compensating the q/k fp8 scales), r=exp(G-M) folds into the per-row
normalizer; 1-D scans run on the DVE.
"""

import math
import threading

import numpy as np
import ml_dtypes

import concourse.bass as bass  # noqa: F401
import concourse.mybir as mybir
import concourse.tile as tile
from concourse import bacc
from concourse.bass_types import AP as APc
from concourse.masks import make_identity
from concourse.bass_utils import run_bass_kernel_spmd

P = 128
B, S, E = 8, 1024, 512
L, H, KC, F = 2, 4, 4, 128
I = 2 * E
DH = I // H      # 256
EC = E // P      # 4
IC = I // P      # 8
ST = S // P      # 8
HV = 2           # halves of 512 along seq
EPS = 1e-5
NCORES = 8
PAD = 16         # xm left pad (16B-aligned chunk stride for DoubleRow)
VW = DH + 4      # padded v row (260: 16B-aligned st stride)

f32 = mybir.dt.float32
bf16 = mybir.dt.bfloat16
e4 = mybir.dt.float8e4
AF = mybir.ActivationFunctionType
OP = mybir.AluOpType
DR = mybir.MatmulPerfMode.DoubleRow
BF = ml_dtypes.bfloat16
E4 = ml_dtypes.float8_e4m3

# activation fp8 storage scales (inputs are deterministic; 2-3x headroom
# below the TRN e4m3 max of 240). Scores/V stay bf16: the u-scaled score
# tile spans ~e^-50..1 and would flush to zero in fp8.
A_Q = 8192.0
A_K = 512.0
A_HF = 4.0                        # hfin fp8 storage scale
DU = math.log(A_Q * A_K)          # decay bias compensating q/k scaling


def _p2(maxval, target=56.0):
    if maxval <= 0:
        return 1.0
    return float(2.0 ** np.floor(np.log2(target / maxval)))


# ---------------------------------------------------------------------------
# host-side weight preparation
# ---------------------------------------------------------------------------

def _prep_weights(inp):
    w = {}
    sc = {}
    scale = DH ** -0.5

    wq = np.asarray(inp["wq"], np.float32) * scale
    wk = np.asarray(inp["wk"], np.float32)
    wv = np.asarray(inp["wv"], np.float32)
    w_i = np.asarray(inp["w_i"], np.float32)
    w_f = np.asarray(inp["w_f"], np.float32)
    wup = np.asarray(inp["w_up"], np.float32)
    wdn = np.asarray(inp["w_down"], np.float32)
    cw = np.asarray(inp["conv_w"], np.float32)

    # per-tensor power-2 fp8 scales (shared across layers: distributions match)
    sc["up"] = _p2(np.abs(wup).max())
    sc["q"] = _p2(np.abs(wq).max())
    sc["k"] = _p2(np.abs(wk).max())
    sc["v"] = _p2(np.abs(wv).max())
    sc["cv"] = _p2(np.abs(cw).max())

    sc["dn"] = _p2(np.abs(wdn).max())

    # streamed m-tile layouts: [L, M, P, C, P] with [l, m, p, c, f] = W[l, c*P+p, m*P+f]
    def mtile(a, mt, ct, s, dt=E4):
        lw = (a * s).reshape(L, ct, P, mt, P).transpose(0, 3, 2, 1, 4)
        return np.ascontiguousarray(lw).astype(dt)

    w["wup"] = mtile(wup, 16, EC, sc["up"])
    w["wq"] = mtile(wq, IC, IC, sc["q"])
    w["wk"] = mtile(wk, IC, IC, sc["k"])
    w["wdown"] = mtile(wdn, EC, IC, sc["dn"])

    # wv: [L, half, P, C, 512] with [l,h,p,c,n] = wv[l, c*P+p, h*512+n]
    wvl = (wv * sc["v"]).reshape(L, IC, P, HV, 512).transpose(0, 3, 2, 1, 4)
    w["wv"] = np.ascontiguousarray(wvl).astype(E4)

    # depthwise-conv diag blocks: [L, P, C, KC, P], [l,p,c,j,f] = conv_w[l,j,c*P+p] iff f==p
    cwr = (cw * sc["cv"]).reshape(L, KC, IC, P)  # [l,j,c,p]
    cd = np.zeros((L, P, IC, KC, P), np.float32)
    rng = np.arange(P)
    cd[:, rng, :, :, rng] = cwr.transpose(3, 0, 2, 1)  # -> [p, l, c, j]
    w["wconv"] = cd.astype(E4)

    # fused gate weights: [ipre|fpre](8) = [c_act | x_m](2I) @ Wif + b
    wif = np.zeros((L, 2 * I, 8), np.float32)
    for l in range(L):
        wif[l, :I, 0:4] = (wq[l] / scale) @ w_i[l, :I] + wk[l] @ w_i[l, I:2 * I]
        wif[l, :I, 4:8] = (wq[l] / scale) @ w_f[l, :I] + wk[l] @ w_f[l, I:2 * I]
        wif[l, I:, 0:4] = wv[l] @ w_i[l, 2 * I:]
        wif[l, I:, 4:8] = wv[l] @ w_f[l, 2 * I:]
    sc["if"] = _p2(np.abs(wif).max())
    # pad last dim to 16 so the DoubleRow Ko stride is 16B-aligned
    wif16 = np.zeros((L, 16, P, 16), np.float32)
    wif16[:, :, :, 0:8] = (wif * sc["if"]).reshape(L, 16, P, 8)
    w["wif"] = np.ascontiguousarray(wif16.transpose(0, 2, 1, 3)).astype(E4)

    w["bif"] = np.ascontiguousarray(
        np.concatenate([np.asarray(inp["b_i"], np.float32),
                        np.asarray(inp["b_f"], np.float32)], axis=1).T)  # [8, L]

    # final projection
    w["wfin"] = np.ascontiguousarray(
        np.asarray(inp["w_fin"], np.float32).reshape(EC, P, F).transpose(1, 0, 2)
    ).astype(BF)  # [P, EC, F]
    w["bfin"] = np.asarray(inp["b_fin"], np.float32).reshape(1, F).copy()

    def cols(a, c):  # [L, c*P] -> [P, L, c]
        return np.ascontiguousarray(
            np.asarray(a, np.float32).reshape(L, c, P).transpose(2, 0, 1))

    w["lng"] = cols(inp["ln_g"], EC)
    w["lnb"] = cols(inp["ln_b"], EC)
    w["convb"] = cols(inp["conv_b"], IC)
    # hfin is stored fp8 at scale A_HF: fold A_HF into the gn-gamma and skip
    # constants (exactly compensated by the down drain scale)
    w["skip"] = cols(np.asarray(inp["skip"], np.float32) * A_HF, IC)
    w["gng"] = cols(np.asarray(inp["gn_g"], np.float32) * A_HF, IC)
    w["bdown"] = cols(inp["b_down"], EC)

    # causal staircase mask [P, 896]: mask[s, j] = 1 iff j >= s + 384
    jj = np.arange(896)[None, :]
    ss = np.arange(P)[:, None]
    w["cmask"] = (jj >= ss + 384).astype(BF)
    return w, sc


# ---------------------------------------------------------------------------
# device kernel
# ---------------------------------------------------------------------------

def build_nc(sc, cfg=None, repeat=1):
    base_cfg = dict(scores_bufs=2, big_bufs=4, sv_bufs=3, wv_bufs=2,
                    w_bufs=2, ht_bufs=6)
    base_cfg.update(cfg or {})
    cfg = base_cfg
    nc = bacc.Bacc("TRN2", target_bir_lowering=False, debug=False,
                   num_devices=NCORES)

    x_d = nc.declare_dram_parameter("x", [S, E], f32, isOutput=False)
    wup_d = nc.declare_dram_parameter("wup", [L, 16, P, EC, P], e4, isOutput=False)
    wq_d = nc.declare_dram_parameter("wq", [L, IC, P, IC, P], e4, isOutput=False)
    wk_d = nc.declare_dram_parameter("wk", [L, IC, P, IC, P], e4, isOutput=False)
    wv_d = nc.declare_dram_parameter("wv", [L, HV, P, IC, 512], e4, isOutput=False)
    wdown_d = nc.declare_dram_parameter("wdown", [L, EC, P, IC, P], e4, isOutput=False)
    wconv_d = nc.declare_dram_parameter("wconv", [L, P, IC, KC, P], e4, isOutput=False)
    wif_d = nc.declare_dram_parameter("wif", [L, P, 16, 16], e4, isOutput=False)
    bif_d = nc.declare_dram_parameter("bif", [8, L], f32, isOutput=False)
    wfin_d = nc.declare_dram_parameter("wfin", [P, EC, F], bf16, isOutput=False)
    bfin_d = nc.declare_dram_parameter("bfin", [1, F], f32, isOutput=False)
    lng_d = nc.declare_dram_parameter("lng", [P, L, EC], f32, isOutput=False)
    lnb_d = nc.declare_dram_parameter("lnb", [P, L, EC], f32, isOutput=False)
    convb_d = nc.declare_dram_parameter("convb", [P, L, IC], f32, isOutput=False)
    skip_d = nc.declare_dram_parameter("skip", [P, L, IC], f32, isOutput=False)
    gng_d = nc.declare_dram_parameter("gng", [P, L, IC], f32, isOutput=False)
    bdown_d = nc.declare_dram_parameter("bdown", [P, L, EC], f32, isOutput=False)
    cmask_d = nc.declare_dram_parameter("cmask", [P, 896], bf16, isOutput=False)
    y_d = nc.declare_dram_parameter("y", [S, F], f32, isOutput=True)

    r_up = 1.0 / sc["up"]
    r_cv = 1.0 / sc["cv"]
    r_if = 1.0 / sc["if"]
    s_q = A_Q / sc["q"]
    s_k = A_K / sc["k"]
    s_v = 1.0 / sc["v"]
    r_dn = 1.0 / (sc["dn"] * A_HF)

    with tile.TileContext(nc) as tc:
        const = tc.alloc_tile_pool(name="const", bufs=1)
        rpool = tc.alloc_tile_pool(name="rpool", bufs=1)
        act = tc.alloc_tile_pool(name="act", bufs=1)
        wstream = tc.alloc_tile_pool(name="wstream", bufs=cfg["w_bufs"])
        smalls = tc.alloc_tile_pool(name="smalls", bufs=1)
        ps_big = tc.alloc_tile_pool(name="ps_big", bufs=cfg["big_bufs"], space="PSUM")
        ps_sv = tc.alloc_tile_pool(name="ps_sv", bufs=cfg["sv_bufs"], space="PSUM")
        ps_sm = tc.alloc_tile_pool(name="ps_sm", bufs=1, space="PSUM")

        # ---- constants
        id_f32 = const.tile([P, P], f32)
        make_identity(nc, id_f32)
        id_bf = const.tile([P, P], bf16)
        make_identity(nc, id_bf)
        ones_bf = const.tile([P, P], bf16)
        nc.vector.memset(ones_bf, 1.0 / E)
        cmask = const.tile([P, 896], bf16)
        nc.sync.dma_start(out=cmask, in_=cmask_d[:, :])
        lng = const.tile([P, L, EC], f32)
        nc.sync.dma_start(out=lng, in_=lng_d[:, :, :])
        lnb = const.tile([P, L, EC], f32)
        nc.sync.dma_start(out=lnb, in_=lnb_d[:, :, :])
        convb = const.tile([P, L, IC], f32)
        nc.sync.dma_start(out=convb, in_=convb_d[:, :, :])
        skipc = const.tile([P, L, IC], f32)
        nc.sync.dma_start(out=skipc, in_=skip_d[:, :, :])
        gng = const.tile([P, L, IC], f32)
        nc.sync.dma_start(out=gng, in_=gng_d[:, :, :])
        bdown = const.tile([P, L, EC], f32)
        nc.sync.dma_start(out=bdown, in_=bdown_d[:, :, :])
        bif = const.tile([8, L], f32)
        nc.sync.dma_start(out=bif, in_=bif_d[:, :])
        wfin = const.tile([P, EC, F], bf16)
        nc.sync.dma_start(out=wfin, in_=wfin_d[:, :, :])
        bfin = const.tile([P, F], f32)
        nc.gpsimd.dma_start(out=bfin, in_=bfin_d.ap().to_broadcast([P, F]))
        eps_col = const.tile([P, 1], f32)
        nc.vector.memset(eps_col, EPS)
        one_col = const.tile([P, 1], f32)
        nc.vector.memset(one_col, 1.0)

        # ---- residual, feature-major [P, EC, S] fp32, updated in place
        r_feat = rpool.tile([P, EC, S], f32)

        # PE warm-up burst: ~5us of junk matmuls overlapping the x DMA so the
        # HAM clock-gate reaches 8/8 before the first real matmul.
        ps_warm = ps_sm.tile([P, 512], f32, tag="sm", name="ps_warm")
        for _w in range(20):
            nc.tensor.matmul(ps_warm, ones_bf, cmask[:, 0:512],
                             start=(_w == 0), stop=(_w == 19))

        def overlap2(base2d, width):
            """[P, *] AP -> [P, 2, width] with unit pair step (conv windows)."""
            pstride, pcount = base2d.ap[0]
            return APc(base2d.tensor, base2d.offset,
                       [[pstride, pcount], [1, 2], [1, width]])

        for _rep in range(repeat):
            # load x seq-major and transpose into r_feat
            xseq = act.tile([P, ST, E], f32, tag="cact", name="xseq")
            x_r = x_d.ap().rearrange("(t p) e -> p t e", p=P)
            for tt in range(ST):
                nc.sync.dma_start(out=xseq[:, tt:tt + 1, :],
                                  in_=x_r[:, tt:tt + 1, :])
            for c in range(EC):
                for h in range(HV):
                    pst = ps_big.tile([P, 512], f32, tag="big", name="ps_xT")
                    for k in range(4):
                        st = h * 4 + k
                        nc.tensor.transpose(
                            out=pst[:, k * P:(k + 1) * P],
                            in_=xseq[:, st, c * P:(c + 1) * P],
                            identity=id_f32)
                    nc.vector.tensor_copy(out=r_feat[:, c, h * 512:(h + 1) * 512],
                                          in_=pst)

            # ---- layernorm for one 512-seq half (stats via ones-lhsT matmuls);
            # pipelined into the PREVIOUS layer's attention tail: half h only
            # needs that half's down-proj done.
            xn_t = {}

            def ln_half(l, h):
                sl = slice(h * 512, (h + 1) * 512)
                rbh = act.tile([P, EC, 512], bf16, tag="q", name="rb")
                sqh = act.tile([P, EC, 512], bf16, tag="sq", name="sq", bufs=2)
                xnh = act.tile([P, EC, 512], e4, tag="scoresB", name="xn",
                               bufs=cfg["scores_bufs"])
                xn_t[h] = xnh
                rstd_b = smalls.tile([P, 512], bf16, name="rstd_b", tag="rstd",
                                     bufs=2)
                rstd_f = smalls.tile([P, 512], f32, name="rstd_f", tag="rstdf")
                tvar = smalls.tile([P, 512], f32, name="tvar", tag="tvar")
                for c in range(EC):
                    # split the bf16 casts across both engines: at the layer
                    # boundary each queue is busy with attention-tail work
                    if c % 2 == 0:
                        nc.scalar.activation(out=rbh[:, c, :],
                                             in_=r_feat[:, c, sl], func=AF.Copy)
                    else:
                        nc.vector.tensor_copy(out=rbh[:, c, :],
                                              in_=r_feat[:, c, sl])
                for c in range(EC):
                    nc.vector.tensor_mul(out=sqh[:, c, :], in0=r_feat[:, c, sl],
                                         in1=r_feat[:, c, sl])
                # ones_bf holds 1/E: psums are mu and E[x^2] directly
                ps_sum = ps_sv.tile([P, 512], f32, tag="sv", name="ps_lnsum")
                ps_sq = ps_sv.tile([P, 512], f32, tag="sv", name="ps_lnsq")
                for c in range(EC):
                    nc.tensor.matmul(ps_sum, ones_bf, rbh[:, c, :],
                                     start=(c == 0), stop=(c == EC - 1))
                for c in range(EC):
                    nc.tensor.matmul(ps_sq, ones_bf, sqh[:, c, :],
                                     start=(c == 0), stop=(c == EC - 1))
                nc.scalar.activation(out=tvar, in_=ps_sum, func=AF.Square)
                nc.vector.tensor_sub(out=tvar, in0=ps_sq, in1=tvar)
                nc.scalar.activation(out=tvar, in_=tvar, func=AF.Sqrt,
                                     bias=eps_col)
                with nc.allow_low_precision(reason="approx LN rstd"):
                    nc.vector.reciprocal_approx_fast(out=rstd_f, in_=tvar)
                    nc.vector.tensor_copy(out=rstd_b, in_=rstd_f)
                for c in range(EC):
                    nc.vector.tensor_sub(out=sqh[:, c, :],
                                         in0=r_feat[:, c, sl], in1=ps_sum)
                    nc.vector.tensor_mul(out=sqh[:, c, :], in0=sqh[:, c, :],
                                         in1=rstd_b)
                    nc.scalar.activation(out=xnh[:, c, :], in_=sqh[:, c, :],
                                         func=AF.Identity,
                                         scale=lng[:, l, c:c + 1],
                                         bias=lnb[:, l, c:c + 1])

            ln_half(0, 0)
            ln_half(0, 1)

            # ================= per-block =================
            for l in range(L):
                # HAM keep-warm filler across the layer-boundary latency chain
                if l > 0:
                    ps_w = ps_sm.tile([P, 512], f32, tag="sm", name="ps_warm2")
                    for _w in range(6):
                        nc.tensor.matmul(ps_w, ones_bf, cmask[:, 0:512],
                                         start=(_w == 0), stop=(_w == 5))
                # ---------- up projection (fp8 DoubleRow) ----------
                # m 0..7 -> x_m (PAD zero pad cols), m 8..15 -> z -> silu -> sz
                xm = act.tile([P, IC, S + PAD], e4, tag="xm", name="xm")
                nc.vector.memset(xm[:, :, 0:PAD], 0.0)
                sz = act.tile([P, IC, S], bf16, tag="sz", name="sz")
                for wave in range(2):
                    upw = []
                    for mi in range(8):
                        wt = wstream.tile([P, EC, P], e4, tag="wup",
                                          name="wup_t", bufs=8)
                        nc.sync.dma_start(out=wt, in_=wup_d[l, wave * 8 + mi])
                        upw.append(wt)
                    for h in range(HV):
                        for mi in range(8):
                            m = wave * 8 + mi
                            ps = ps_big.tile([P, 512], f32, tag="big", name="ps_up")
                            for c2 in range(EC // 2):
                                nc.tensor.matmul(
                                    ps, upw[mi][:, 2 * c2:2 * c2 + 2, :],
                                    xn_t[h][:, 2 * c2:2 * c2 + 2, :],
                                    start=(c2 == 0), stop=(c2 == EC // 2 - 1),
                                    perf_mode=DR)
                            if m < IC:
                                if h == 0:
                                    nc.vector.tensor_scalar_mul(
                                        out=xm[:, m,
                                               PAD + h * 512:PAD + (h + 1) * 512],
                                        in0=ps, scalar1=r_up)
                                else:
                                    nc.scalar.activation(
                                        out=xm[:, m,
                                               PAD + h * 512:PAD + (h + 1) * 512],
                                        in_=ps, func=AF.Copy, scale=r_up)
                            else:
                                nc.scalar.activation(
                                    out=sz[:, m - IC, h * 512:(h + 1) * 512],
                                    in_=ps, func=AF.Silu, scale=r_up)

                # ---------- causal depthwise conv (diag DR matmuls) + silu ----------
                cact = act.tile([P, IC, S], e4, tag="cact", name="cact")
                csk_t = act.tile([P, IC, S], e4, tag="csk", name="csk_t")
                for c in range(IC):
                    wcv = wstream.tile([P, KC, P], e4, tag="wconv", name="wconv_t",
                                       bufs=2)
                    nc.sync.dma_start(out=wcv, in_=wconv_d[l, :, c])
                    for h in range(HV):
                        ps = ps_big.tile([P, 512], f32, tag="big", name="ps_cv")
                        for j2 in range(KC // 2):
                            base = xm[:, c, PAD - 3 + 2 * j2 + h * 512:]
                            nc.tensor.matmul(
                                ps, wcv[:, 2 * j2:2 * j2 + 2, :],
                                overlap2(base, 512),
                                start=(j2 == 0), stop=(j2 == KC // 2 - 1),
                                perf_mode=DR)
                        nc.scalar.activation(
                            out=cact[:, c, h * 512:(h + 1) * 512], in_=ps,
                            func=AF.Silu, scale=r_cv, bias=convb[:, l, c:c + 1])
                    # skip-path copy (skip pre-scaled by A_HF on host), off the
                    # attention critical path
                    nc.vector.tensor_scalar_mul(out=csk_t[:, c, :],
                                                in0=cact[:, c, :],
                                                scalar1=skipc[:, l, c:c + 1])

                # ---------- gate pre-activations + scans ----------
                wif = wstream.tile([P, 16, 16], e4, tag="wif", name="wif_t")
                nc.sync.dma_start(out=wif, in_=wif_d[l])
                # compute-engine APs must start at partition 0/32/64/96, so the
                # 1-D gate chain lives in base-0 [4,S]/[8,S] tiles; fpre is
                # extracted from rows 4:8 via an SBUF->SBUF DMA shuffle.
                g8 = smalls.tile([8, S], f32, name="g8")      # 0:4 ipre->a, 4:8 fpre
                f4 = smalls.tile([4, S], f32, name="f4")      # fpre -> sp -> u
                spc4 = smalls.tile([4, S], f32, name="spc4")  # spc -> em
                mr4 = smalls.tile([4, S], f32, name="mr4")    # runmax -> r
                for h in range(HV):
                    psg = ps_sm.tile([8, 512], f32, tag="sm", name="ps_g")
                    for c2 in range(4):
                        nc.tensor.matmul(
                            psg, wif[:, 2 * c2:2 * c2 + 2, 0:8],
                            cact[:, 2 * c2:2 * c2 + 2, h * 512:(h + 1) * 512],
                            start=(c2 == 0), stop=False, perf_mode=DR)
                    for c2 in range(4):
                        nc.tensor.matmul(
                            psg, wif[:, 8 + 2 * c2:10 + 2 * c2, 0:8],
                            xm[:, 2 * c2:2 * c2 + 2,
                               PAD + h * 512:PAD + (h + 1) * 512],
                            start=False, stop=(c2 == 3), perf_mode=DR)
                    nc.vector.tensor_scalar(out=g8[:, h * 512:(h + 1) * 512],
                                            in0=psg, scalar1=r_if,
                                            scalar2=bif[:, l:l + 1],
                                            op0=OP.mult, op1=OP.add)
                nc.sync.dma_start(out=f4, in_=g8[4:8, :])
                # sp = softplus(-fpre) (in place), spc = cumsum(sp)
                nc.scalar.activation(out=f4, in_=f4, func=AF.Exp, scale=-1.0)
                nc.scalar.activation(out=f4, in_=f4, func=AF.Ln, bias=one_col[0:4])
                nc.vector.tensor_tensor_scan(out=spc4, data0=f4, data1=f4,
                                             initial=0.0, op0=OP.add, op1=OP.bypass)
                # a = ipre + spc (overwrites ipre), Mr = runmax(a), G = Mr[-1]
                nc.vector.tensor_add(out=g8[0:4], in0=g8[0:4], in1=spc4)
                nc.vector.tensor_tensor_scan(out=mr4, data0=g8[0:4], data1=g8[0:4],
                                             initial=-3.0e38, op0=OP.max,
                                             op1=OP.bypass)
                gmax = smalls.tile([4, 1], f32, name="gmax")
                ngmax = smalls.tile([4, 1], f32, name="ngmax")
                nc.vector.tensor_copy(out=gmax, in_=mr4[:, S - 1:S])
                nc.vector.tensor_scalar(out=ngmax, in0=gmax, scalar1=-1.0,
                                        scalar2=-DU, op0=OP.mult, op1=OP.add)
                # u' = exp(a-G-du) -> f4 (sp dead); em' = B*exp(spc-Mr) -> spc4;
                # r = exp(G-Mr) -> mr4 in place
                nc.scalar.activation(out=f4, in_=g8[0:4], func=AF.Exp, bias=ngmax)
                nc.vector.tensor_sub(out=spc4, in0=spc4, in1=mr4)
                nc.scalar.activation(out=spc4, in_=spc4, func=AF.Exp)
                nc.scalar.activation(out=mr4, in_=mr4, func=AF.Exp, scale=-1.0,
                                     bias=gmax)

                # ---------- v projection (seq-major, fp8 DR in, bf16 out) ----------
                v_sb = act.tile([P, ST, H, VW], bf16, tag="v", name="v_sb")
                nc.vector.memset(v_sb[:, :, :, DH:DH + 1], 1.0)
                for h in range(HV):
                    wvt = wstream.tile([P, IC, 512], e4, tag="wv", name="wv_t",
                                       bufs=cfg["wv_bufs"])
                    nc.sync.dma_start(out=wvt, in_=wv_d[l, h])
                    for st in range(ST):
                        ps = ps_big.tile([P, 512], f32, tag="big", name="ps_v")
                        for c2 in range(IC // 2):
                            nc.tensor.matmul(
                                ps,
                                xm[:, 2 * c2:2 * c2 + 2,
                                   PAD + st * P:PAD + (st + 1) * P],
                                wvt[:, 2 * c2:2 * c2 + 2, :],
                                start=(c2 == 0), stop=(c2 == IC // 2 - 1),
                                perf_mode=DR)
                        nc.scalar.activation(
                            out=v_sb[:, st, 2 * h:2 * h + 2, 0:DH],
                            in_=ps.rearrange("p (a b) -> p a b", a=2),
                            func=AF.Copy, scale=s_v)

                # ---------- q/k (per head) + attention, software-pipelined ----------
                # Each head's attention drains + normalizer chain run on
                # DVE/ACT while the PE computes the NEXT head's q/k
                # projections; transposes back to feature-major lag one
                # attention step further; the half's down-proj closes it out.
                q_f = act.tile([P, IC, S], e4, tag="q", name="q_f")
                k_f = act.tile([P, IC, S], e4, tag="k", name="k_f")
                hgn = act.tile([P, ST, S], bf16, tag="xn", name="hgn")
                hfin = act.tile([P, IC, S], e4, tag="xm", name="hfin")
                useq = smalls.tile([P, ST, 12], f32, name="useq")
                dnw = []
                for m in range(EC):
                    wt = wstream.tile([P, IC, P], e4, tag="wdown", name="wdown_t",
                                      bufs=4)
                    nc.sync.dma_start(out=wt, in_=wdown_d[l, m])
                    dnw.append(wt)

                def qk_head(hd):
                    for dst, wdrm, wtag, sdr, dve in ((q_f, wq_d, "wq", s_q, True),
                                                      (k_f, wk_d, "wk", s_k, False)):
                        for m in (2 * hd, 2 * hd + 1):
                            wt = wstream.tile([P, IC, P], e4, tag=wtag,
                                              name="wqk_t")
                            nc.sync.dma_start(out=wt, in_=wdrm[l, m])
                            for h in range(HV):
                                ps = ps_big.tile([P, 512], f32, tag="big",
                                                 name="ps_qk")
                                for c2 in range(IC // 2):
                                    nc.tensor.matmul(
                                        ps, wt[:, 2 * c2:2 * c2 + 2, :],
                                        cact[:, 2 * c2:2 * c2 + 2,
                                             h * 512:(h + 1) * 512],
                                        start=(c2 == 0),
                                        stop=(c2 == IC // 2 - 1), perf_mode=DR)
                                if dve:
                                    nc.vector.tensor_scalar_mul(
                                        out=dst[:, m, h * 512:(h + 1) * 512],
                                        in0=ps, scalar1=sdr)
                                else:
                                    nc.scalar.activation(
                                        out=dst[:, m, h * 512:(h + 1) * 512],
                                        in_=ps, func=AF.Copy, scale=sdr)

                def useq_t():
                    # u'/r/em to seq-major: useq[:, c, 0:4]=u, 4:8=r, 8:12=em
                    for c in range(ST):
                        for qi, srct in enumerate((f4, mr4, spc4)):
                            pst = ps_sv.tile([P, 4], f32, tag="sv",
                                             name="ps_useq")
                            nc.tensor.transpose(out=pst,
                                                in_=srct[:, c * P:(c + 1) * P],
                                                identity=id_f32[0:4, 0:4])
                            nc.vector.tensor_copy(
                                out=useq[:, c, qi * 4:qi * 4 + 4], in_=pst)

                def attn(hd, tc_):
                    scores = act.tile(
                        [P, 4 * (tc_ + 1), 512], bf16,
                        tag=("scoresA", "scoresB")[tc_], name="scores",
                        bufs=(3 if tc_ == 0 else cfg["scores_bufs"]))
                    ncc = 4 * (tc_ + 1)
                    for cc in range(ncc):
                        d = cc * P - tc_ * 512
                        d0 = max(d, 0)  # first needed t_local column
                        nw = 512 - d0
                        ps = ps_big.tile([P, 512], f32, tag="big", name="ps_qkT")
                        nc.tensor.matmul(
                            ps[:, 0:nw],
                            k_f[:, 2 * hd:2 * hd + 2, cc * P:(cc + 1) * P],
                            q_f[:, 2 * hd:2 * hd + 2,
                                tc_ * 512 + d0:(tc_ + 1) * 512],
                            start=True, stop=True, perf_mode=DR)
                        ucol = useq[:, cc, hd:hd + 1]
                        if d >= 0:
                            # diagonal corner tile gets the triangular mask;
                            # the fully-causal remainder is a plain u-scaled
                            # copy; t_local < d is never read by sv.
                            nc.vector.scalar_tensor_tensor(
                                out=scores[:, cc, d:d + P],
                                in0=ps[:, 0:P], scalar=ucol,
                                in1=cmask[:, 384:512],
                                op0=OP.mult, op1=OP.mult)
                            if d + P < 512:
                                nc.scalar.activation(
                                    out=scores[:, cc, d + P:512],
                                    in_=ps[:, P:nw],
                                    func=AF.Copy, scale=ucol)
                        else:
                            nc.scalar.activation(out=scores[:, cc, :], in_=ps,
                                                 func=AF.Copy, scale=ucol)
                    # scores @ v_aug per 128-row tile; normalizer + group
                    # norm batched over the 4 row-tiles of this t-chunk
                    hts = []
                    mv = smalls.tile([P, 4, 2], f32, name="mv", tag="mv", bufs=4)
                    bns = smalls.tile([P, 4, 6], f32, name="bns", tag="bns",
                                      bufs=4)
                    st_g = smalls.tile([P, 7, 4], f32, name="st_g", tag="st_g",
                                       bufs=4)
                    for ti in range(4):
                        t = tc_ * 4 + ti
                        pso = ps_sv.tile([P, DH + 1], f32, tag="sv", name="ps_sv")
                        for cc in range(t + 1):
                            nc.tensor.matmul(
                                pso,
                                scores[:, cc, ti * P:(ti + 1) * P],
                                v_sb[:, cc, hd, 0:DH + 1],
                                start=(cc == 0), stop=(cc == t))
                        rcol = useq[:, t, 4 + hd:5 + hd]
                        # O~ = r * O_raw (bounded);  O~[:,DH] = r*S_raw
                        ht = smalls.tile([P, DH + 1], bf16, name="ht", tag="ht",
                                         bufs=cfg["ht_bufs"])
                        nc.scalar.activation(out=ht, in_=pso, func=AF.Copy,
                                             scale=rcol)
                        hts.append(ht)
                        # |r*S_raw| straight off ht (no gpsimd, no ACT Abs)
                        nc.vector.scalar_tensor_tensor(
                            out=st_g[:, 1, ti:ti + 1], in0=ht[:, DH:DH + 1],
                            scalar=-1.0, in1=ht[:, DH:DH + 1],
                            op0=OP.mult, op1=OP.max)
                        nc.vector.bn_stats(out=bns[:, ti, :], in_=ht[:, 0:DH])
                        nc.vector.bn_aggr(out=mv[:, ti, :], in_=bns[:, ti, :])
                    em4 = useq[:, tc_ * 4:tc_ * 4 + 4, 8 + hd]
                    var4 = mv[:, :, 1]
                    # n = max(|r*S_raw|, em);  phi = rsqrt(var + eps*n^2)
                    nc.vector.tensor_max(out=st_g[:, 2, :], in0=st_g[:, 1, :],
                                         in1=em4)
                    nc.vector.tensor_mul(out=st_g[:, 3, :], in0=st_g[:, 2, :],
                                         in1=st_g[:, 2, :])
                    nc.vector.scalar_tensor_tensor(
                        out=st_g[:, 3, :], in0=st_g[:, 3, :], scalar=EPS,
                        in1=var4, op0=OP.mult, op1=OP.add)
                    nc.scalar.activation(out=st_g[:, 4, :], in_=st_g[:, 3, :],
                                         func=AF.Sqrt)
                    with nc.allow_low_precision(reason="GN phi"):
                        nc.vector.reciprocal(out=st_g[:, 5, :],
                                             in_=st_g[:, 4, :])  # phi
                    for ti in range(4):
                        t = tc_ * 4 + ti
                        nc.vector.tensor_scalar(
                            out=hgn[:, t, hd * DH:(hd + 1) * DH],
                            in0=hts[ti][:, 0:DH],
                            scalar1=mv[:, ti, 0:1],
                            scalar2=st_g[:, 5, ti:ti + 1],
                            op0=OP.subtract, op1=OP.mult)

                def h_t(hd, tc_):
                    # feature-major transpose + residual mix for this head/half
                    for c in (2 * hd, 2 * hd + 1):
                        pst = ps_big.tile([P, 512], bf16, tag="big", name="ps_hT")
                        for k in range(4):
                            t = tc_ * 4 + k
                            nc.tensor.transpose(
                                out=pst[:, k * P:(k + 1) * P],
                                in_=hgn[:, t, c * P:(c + 1) * P],
                                identity=id_bf)
                        post = smalls.tile([P, 512], bf16, name="post",
                                           tag="post", bufs=2)
                        nc.vector.scalar_tensor_tensor(
                            out=post, in0=pst, scalar=gng[:, l, c:c + 1],
                            in1=csk_t[:, c, tc_ * 512:(tc_ + 1) * 512],
                            op0=OP.mult, op1=OP.add)
                        nc.vector.tensor_mul(
                            out=hfin[:, c, tc_ * 512:(tc_ + 1) * 512],
                            in0=post,
                            in1=sz[:, c, tc_ * 512:(tc_ + 1) * 512])

                def down(tc_):
                    # down projection for this half + residual (fp8 DR; bdown
                    # applied in the psum-descale copy, residual add on DVE)
                    for m in range(EC):
                        ps = ps_big.tile([P, 512], f32, tag="big", name="ps_dn")
                        for c2 in range(IC // 2):
                            nc.tensor.matmul(
                                ps, dnw[m][:, 2 * c2:2 * c2 + 2, :],
                                hfin[:, 2 * c2:2 * c2 + 2,
                                     tc_ * 512:(tc_ + 1) * 512],
                                start=(c2 == 0), stop=(c2 == IC // 2 - 1),
                                perf_mode=DR)
                        dtmp = smalls.tile([P, 512], bf16, name="dtmp",
                                           tag="post", bufs=2)
                        nc.scalar.activation(out=dtmp, in_=ps, func=AF.Identity,
                                             scale=r_dn,
                                             bias=bdown[:, l, m:m + 1])
                        nc.vector.tensor_add(
                            out=r_feat[:, m, tc_ * 512:(tc_ + 1) * 512],
                            in0=r_feat[:, m, tc_ * 512:(tc_ + 1) * 512],
                            in1=dtmp)

                qk_head(0)
                qk_head(1)
                useq_t()
                attn(0, 0)
                attn(0, 1)
                qk_head(2)
                attn(1, 0)
                attn(1, 1)
                qk_head(3)
                h_t(0, 0)
                h_t(0, 1)
                attn(2, 0)
                attn(2, 1)
                h_t(1, 0)
                h_t(1, 1)
                attn(3, 0)
                h_t(2, 0)
                h_t(2, 1)
                attn(3, 1)
                h_t(3, 0)
                down(0)
                h_t(3, 1)
                if l + 1 < L:
                    ln_half(l + 1, 0)
                down(1)
                if l + 1 < L:
                    ln_half(l + 1, 1)

            # ================= final projection =================
            r_bf = act.tile([P, EC, S], bf16, tag="xn", name="r_bf")
            for h in range(HV):
                for c in range(EC):
                    nc.vector.tensor_copy(
                        out=r_bf[:, c, h * 512:(h + 1) * 512],
                        in_=r_feat[:, c, h * 512:(h + 1) * 512])
            yout = act.tile([P, ST, F], f32, tag="sz", name="yout")
            for st in range(ST):
                ps = ps_big.tile([P, F], f32, tag="big", name="ps_fin")
                for c in range(EC):
                    nc.tensor.matmul(ps, r_bf[:, c, st * P:(st + 1) * P],
                                     wfin[:, c, :],
                                     start=(c == 0), stop=(c == EC - 1))
                nc.vector.tensor_add(out=yout[:, st, :], in0=ps, in1=bfin)
            nc.sync.dma_start(out=y_d.ap().rearrange("(t p) f -> p t f", p=P),
                              in_=yout)

        for pool in (ps_sm, ps_sv, ps_big, smalls, wstream, act, rpool, const):
            pool.release()

    nc.compile()
    return nc


# ---------------------------------------------------------------------------
# entry point
# ---------------------------------------------------------------------------

_lock = threading.Lock()
_nc = None
_sc = None


def _get_nc(inputs=None):
    global _nc, _sc
    with _lock:
        if _nc is None:
            assert inputs is not None
            _, _sc = _prep_weights(inputs)
            _nc = build_nc(_sc)
    return _nc


def _in_maps(inputs):
    w, _ = _prep_weights(inputs)
    x = np.asarray(inputs["x"], np.float32)
    in_maps = []
    for b in range(NCORES):
        m = {"x": np.ascontiguousarray(x[b])}
        m.update(w)
        in_maps.append(m)
    return in_maps


def kernel(**inputs):
    nc = _get_nc(inputs)
    res = run_bass_kernel_spmd(nc, _in_maps(inputs),
                               core_ids=list(range(NCORES)))
    out = np.stack([res.results[b]["y"] for b in range(NCORES)], axis=0)
    return out.astype(np.float32)
